# revision 15
# baseline (speedup 1.0000x reference)
"""Fused transformer decoder layer (self-attn + cross-attn + FFN, 3 LayerNorms)
for Trainium2, SPMD across 8 NeuronCores.

Sharding: 2 cores per batch element (B=4). Each core owns 512 query rows of
its batch element, picked as four 128-row blocks interleaved so the causal
self-attention work is balanced across the pair ({0,3,4,7} / {1,2,5,6}).
K/V projections are computed redundantly per core (no collectives needed).

On-device layout: activations are kept feature-major ("transposed", [D, rows])
so every linear layer uses the weight matrices exactly as stored:
    out^T [Dout, r] = matmul(lhsT=W[D, Dout]-tile, rhs=act^T[D, r]-tile).
Attention scores are computed transposed (scores^T[k, q] = K·Q^T per head);
softmax denominators are harvested by augmenting V with 64 all-ones columns,
which lands the per-query sums partition-replicated next to the attnV output.

v2: everything streams as bf16 (activations, weights, probs); PSUM/LN-stats/
softmax-normalization stay fp32. All weights are packed host-side into ONE
bf16 DRAM tensor laid out in the exact [slab, partition, free] tile order the
kernel consumes (contiguous DMA slabs), and all bias/gamma vectors into one
fp32 tensor: 6 kernel operands total, 2 orders of magnitude fewer startup
DMA issues than v1.
"""

import numpy as np

import concourse.bacc as bacc
import concourse.bass as bass
import concourse.mybir as mybir
import concourse.tile as tile
from concourse.bass_utils import run_bass_kernel_spmd

F32 = mybir.dt.float32
F16 = mybir.dt.float16
AF = mybir.ActivationFunctionType
ALU = mybir.AluOpType

B, S, D, DFF, H = 4, 1024, 1024, 4096, 16
R = 512                   # query rows per core
NK = D // 128             # 8 k-tiles over D
NP = H // 2               # 8 head pairs
NM2 = DFF // 128          # 32 m-tiles over DFF
EPS = 1e-3
NEG = -30000.0
BLOCKS = [[0, 3, 4, 7], [1, 2, 5, 6]]   # 128-row q-blocks per half-core

# wpack unit layout: 128 units of [128, 1024] bf16 (see _pack_weights)
#   units  0..31 : attn1  (wv 8u, wq 8u, wk 8u, wo 8u)
#   units 32..63 : attn2  (same)
#   units 64..95 : w_ff1  (32 slabs)
#   units 96..127: w_ff2  (32 slabs, [m][q] order)
NU = 128

# biaspack columns (fp32 [128, 136]):
_BCOLS = ("bq1", "bk1", "bo1", "bq2", "bk2", "bo2", "b_ff2",
          "g1", "be1", "g2", "be2", "g3", "be3")
_BOFF = {nm: 8 * i for i, nm in enumerate(_BCOLS)}
_BOFF["b_ff1"] = 8 * len(_BCOLS)          # 32 cols
BIAS_W = 136  # 13*8 + 32

_NC_CACHE = {}


def _ln(nc, pools, y, gcol, bcol, out_tiles, ones128):
    """LayerNorm over the partition (feature) axis of 8 [128, R] tiles."""
    ps, tmp = pools["psum"], pools["lntmp"]
    pssum = ps.tile([128, R], F32, tag="ps_gen", name="ln_ps_sum")
    for m in range(NK):
        nc.tensor.matmul(pssum, ones128, y[m], start=(m == 0), stop=(m == NK - 1))
    pssq = ps.tile([128, R], F32, tag="ps_gen", name="ln_ps_sq")
    for m in range(NK):
        sq = pools["sq"].tile([128, R], F16, tag="ln_sqt", name="ln_sqt")
        nc.scalar.activation(sq, y[m], AF.Square)
        nc.tensor.matmul(pssq, ones128, sq, start=(m == 0), stop=(m == NK - 1))
    mean = tmp.tile([128, R], F32, tag="ln_mean", name="ln_mean")
    nc.vector.tensor_scalar_mul(mean, pssum, 1.0 / D)
    rv = tmp.tile([128, R], F32, tag="ln_sc", name="ln_rv")
    nc.vector.tensor_scalar_mul(rv, pssq, 1.0 / D)      # E[x^2]
    msq = tmp.tile([128, R], F32, tag="ln_t", name="ln_msq")
    nc.vector.tensor_mul(msq, mean, mean)
    nc.vector.tensor_sub(rv, rv, msq)                   # var
    nc.scalar.activation(rv, rv, AF.Sqrt, bias=pools["epsc"][:, 0:1])
    nc.vector.reciprocal(rv, rv)                        # rstd (broadcast)
    nc.vector.tensor_mul(mean, mean, rv)                # mean*rstd (broadcast)
    for m in range(NK):
        t = tmp.tile([128, R], F32, tag="ln_t", name="ln_t")
        nc.vector.tensor_mul(t, y[m], rv)
        nc.vector.tensor_sub(t, t, mean)
        nc.vector.tensor_scalar(out_tiles[m], t, gcol[:, m:m + 1], bcol[:, m:m + 1],
                                ALU.mult, ALU.add)


def _attention(nc, pools, q_src, kv_src, resid, wunits, wv_view, u0,
               bqc, bkc, bvrow, boc, mask_sb, y_out, ones_row):
    """One multi-head attention block + residual; writes pre-LN y_out tiles.

    wunits: [NU, 128, 1024] bf16 view of wpack; wv_view: [NU//2, 128, 2048]
    view of the same; u0: first unit of this attention's 32-unit region
    (wv 8u, wq 8u, wk 8u, wo 8u)."""
    wpool, ps, tmp = pools["w"], pools["psum"], pools["atmp"]
    causal = mask_sb is not None
    uq, uk, uo = u0 + 8, u0 + 16, u0 + 24

    # persistent V_aug pair; ones columns written once per attention
    vaugs = [pools["vaug"].tile([128, NK, 256], F16, tag="vaug",
                                name=f"vaug_{i}") for i in range(2)]
    for i in range(2):
        nc.vector.memset(vaugs[i][:, :, 64:192], 1.0)

    nh = []
    for p in range(NP):
        # ---- V for two pairs produced together ----
        # V_aug[:, s, :]: cols [v_h0(64) | ones(128) | v_h1(64)]
        if p % 2 == 0:
            j = p // 2
            wvb = wpool.tile([128, NK, 256], F16, tag="wvb", name=f"wvb_{j}")
            nc.sync.dma_start(
                out=wvb,
                in_=wv_view[u0 // 2 + j].rearrange("p (k c) -> p k c", c=256))
            for s in range(NK):
                psv = ps.tile([128, 256], F32, tag="ps_gen", name="ps_v")
                for k in range(NK):
                    nc.tensor.matmul(psv, kv_src[k][:, s * 128:(s + 1) * 128],
                                     wvb[:, k, :],
                                     start=(k == 0), stop=False)
                # bias row via K=1 matmul: psv[m, c] += 1 * bv[c]
                nc.tensor.matmul(psv, ones_row[0:1, :],
                                 bvrow[0:1, j * 256:(j + 1) * 256],
                                 start=False, stop=True)
                for i in range(2):
                    dst = vaugs[i][:, s, :].rearrange(
                        "p (a c) -> p a c", c=64)[:, 0:4:3, :]
                    srcp = psv[:, i * 128:(i + 1) * 128].rearrange(
                        "p (a c) -> p a c", c=64)
                    nc.vector.tensor_copy(dst, srcp)
        vaug = vaugs[p % 2]

        # ---- K^T and Q^T for this pair only (pipelines across pairs) ----
        wcb = wpool.tile([128, NK, 128], F16, tag="wcb", name=f"wkc_{p}")
        nc.sync.dma_start(out=wcb,
                          in_=wunits[uk + p].rearrange("p (k c) -> p k c", c=128))
        kt_p = pools["kt"].tile([128, S], F16, tag="kt", name=f"kt_{p}")
        for half in range(2):
            psk = ps.tile([128, R], F32, tag="ps_gen", name="ps_k")
            for k in range(NK):
                nc.tensor.matmul(psk, wcb[:, k, :],
                                 kv_src[k][:, half * 512:(half + 1) * 512],
                                 start=(k == 0), stop=(k == NK - 1))
            nc.scalar.activation(kt_p[:, half * 512:(half + 1) * 512], psk,
                                 AF.Identity, bias=bkc[:, p:p + 1])
        wcb = wpool.tile([128, NK, 128], F16, tag="wcb", name=f"wqc_{p}")
        nc.sync.dma_start(out=wcb,
                          in_=wunits[uq + p].rearrange("p (k c) -> p k c", c=128))
        psq = ps.tile([128, R], F32, tag="ps_gen", name="ps_q")
        for k in range(NK):
            nc.tensor.matmul(psq, wcb[:, k, :], q_src[k],
                             start=(k == 0), stop=(k == NK - 1))
        qt_p = pools["qt"].tile([128, R], F16, tag="qt", name=f"qt_{p}")
        nc.scalar.activation(qt_p, psq, AF.Identity, bias=bqc[:, p:p + 1])

        # ---- scores^T, exp, attnV (softmax sums ride along in V_aug ones) ----
        psa0 = pools["psatt"].tile([128, R], F32, tag="ps_att0", name="ps_att0")
        psa1 = pools["psatt"].tile([128, R], F32, tag="ps_att1", name="ps_att1")
        for s in range(NK):
            c0 = 128 * (s // 2) if causal else 0
            n = R - c0
            ssc0 = pools["pssc"].tile([128, R], F32, tag="ps_sc0", name="ps_sc0")
            ssc1 = pools["pssc"].tile([128, R], F32, tag="ps_sc1", name="ps_sc1")
            nc.tensor.matmul(ssc0[:, 0:n], kt_p[0:64, s * 128:(s + 1) * 128],
                             qt_p[0:64, c0:R], start=True, stop=True)
            nc.tensor.matmul(ssc1[:, 0:n], kt_p[64:128, s * 128:(s + 1) * 128],
                             qt_p[64:128, c0:R], start=True, stop=True)
            if causal:
                nc.vector.tensor_add(ssc0[:, 0:128], ssc0[:, 0:128], mask_sb[:, s, :])
                nc.vector.tensor_add(ssc1[:, 0:128], ssc1[:, 0:128], mask_sb[:, s, :])
            e0 = pools["exp"].tile([128, R], F16, tag="e0", name="e0")
            e1 = pools["exp"].tile([128, R], F16, tag="e1", name="e1")
            nc.scalar.activation(e0[:, 0:n], ssc0[:, 0:n], AF.Exp, scale=0.125)
            nc.scalar.activation(e1[:, 0:n], ssc1[:, 0:n], AF.Exp, scale=0.125)
            nc.tensor.matmul(psa0[:, c0:R], vaug[:, s, 0:128], e0[:, 0:n],
                             start=(s == 0), stop=(s == NK - 1), skip_group_check=True)
            nc.tensor.matmul(psa1[:, c0:R], vaug[:, s, 128:256], e1[:, 0:n],
                             start=(s == 0), stop=(s == NK - 1), skip_group_check=True)

        # normalize: head0 out rows 0:64 / sums 64:128; head1 sums 0:64 / out 64:128
        nh_p = pools["nh"].tile([128, R], F16, tag=f"nh_{p}", name=f"nh_{p}")
        inv = tmp.tile([128, R], F32, tag="inv", name="inv")
        invs = tmp.tile([128, R], F32, tag="invs", name="invs")
        nc.vector.reciprocal(inv[64:128, :], psa0[64:128, :])
        nc.sync.dma_start(out=invs[0:64, :], in_=inv[64:128, :])
        nc.vector.tensor_mul(nh_p[0:64, :], psa0[0:64, :], invs[0:64, :])
        nc.vector.reciprocal(inv[0:64, :], psa1[0:64, :])
        nc.sync.dma_start(out=invs[64:128, :], in_=inv[0:64, :])
        nc.vector.tensor_mul(nh_p[64:128, :], psa1[64:128, :], invs[64:128, :])
        nh.append(nh_p)

    # ---- output projection + bias + residual ----
    for m in range(NK):
        wcb = wpool.tile([128, NK, 128], F16, tag="wcb", name=f"woc_{m}")
        nc.sync.dma_start(out=wcb,
                          in_=wunits[uo + m].rearrange("p (k c) -> p k c", c=128))
        pso = ps.tile([128, R], F32, tag="ps_gen", name="ps_o")
        for p in range(NP):
            nc.tensor.matmul(pso, wcb[:, p, :], nh[p],
                             start=(p == 0), stop=(p == NP - 1))
        nc.vector.scalar_tensor_tensor(y_out[m], pso, boc[:, m:m + 1],
                                       resid[m], ALU.add, ALU.add)


def build_nc():
    nc = bacc.Bacc("TRN2", target_bir_lowering=False, debug=False)

    xt = nc.dram_tensor("xt", [D, S], F16, kind="ExternalInput")
    xq = nc.dram_tensor("xq", [D, R], F16, kind="ExternalInput")
    enc = nc.dram_tensor("enc", [D, S], F16, kind="ExternalInput")
    maskst = nc.dram_tensor("maskst", [128, NK, 128], F16, kind="ExternalInput")
    wpack = nc.dram_tensor("wpack", [NU * 128 * 1024], F16, kind="ExternalInput")
    biaspack = nc.dram_tensor("biaspack", [128 * BIAS_W], F32,
                              kind="ExternalInput")
    bvpack = nc.dram_tensor("bvpack", [1, 2048], F16, kind="ExternalInput")
    out_t = nc.dram_tensor("out_t", [D, R], F32, kind="ExternalOutput")

    wunits = wpack.rearrange("(u p x) -> u p x", p=128, x=1024)
    wv_view = wpack.rearrange("(v p y) -> v p y", p=128, y=2048)
    bias2d = biaspack.rearrange("(p w) -> p w", w=BIAS_W)

    from contextlib import ExitStack
    with tile.TileContext(nc) as tc, ExitStack() as ctx:
        pools = {
            "const": ctx.enter_context(tc.tile_pool(name="const", bufs=1)),
            "w": ctx.enter_context(tc.tile_pool(name="wpool", bufs=3)),
            "psum": ctx.enter_context(tc.tile_pool(name="pspool", bufs=2, space="PSUM")),
            "lntmp": ctx.enter_context(tc.tile_pool(name="lntmp", bufs=1)),
            "sq": ctx.enter_context(tc.tile_pool(name="sqpool", bufs=2)),
            "o2p": ctx.enter_context(tc.tile_pool(name="o2pool", bufs=1)),
        }
        const = pools["const"]

        # ================= attention scope =================
        with ExitStack() as actx:
            for nm, bufs, space in (("acts", 1, "SBUF"), ("qt", 2, "SBUF"),
                                    ("kt", 2, "SBUF"), ("vaug", 2, "SBUF"),
                                    ("nh", 1, "SBUF"), ("exp", 2, "SBUF"),
                                    ("atmp", 1, "SBUF"), ("amask", 1, "SBUF"),
                                    ("pssc", 2, "PSUM"), ("psatt", 1, "PSUM")):
                pools[nm] = actx.enter_context(
                    tc.tile_pool(name=nm, bufs=bufs, space=space))
            acts = pools["acts"]

            # activations first: these gate the first matmuls
            xt_sb = [acts.tile([128, S], F16, tag=f"kv_{k}", name=f"xt_{k}")
                     for k in range(NK)]
            xq_sb = [acts.tile([128, R], F16, tag=f"xq_{k}", name=f"xq_{k}")
                     for k in range(NK)]
            for k in range(NK):
                nc.sync.dma_start(out=xt_sb[k], in_=xt[k * 128:(k + 1) * 128, :])
            for k in range(NK):
                nc.sync.dma_start(out=xq_sb[k], in_=xq[k * 128:(k + 1) * 128, :])

            # small constants after the big activation loads
            ones128 = const.tile([128, 128], F16, tag="ones128", name="ones128")
            nc.vector.memset(ones128, 1.0)
            epsc = const.tile([128, 1], F32, tag="epsc", name="epsc")
            nc.vector.memset(epsc, EPS)
            pools["epsc"] = epsc
            biastile = const.tile([128, BIAS_W], F32, tag="biast", name="biast")
            nc.sync.dma_start(out=biastile, in_=bias2d)
            bvrows = const.tile([1, 2048], F16, tag="bvrows", name="bvrows")
            nc.sync.dma_start(out=bvrows, in_=bvpack[:, :])
            bias_cols = {nm: biastile[:, _BOFF[nm]:_BOFF[nm] + 8] for nm in _BCOLS}
            bff1c = biastile[:, _BOFF["b_ff1"]:_BOFF["b_ff1"] + 32]

            mask_sb = pools["amask"].tile([128, NK, 128], F16, tag="mask", name="mask")
            nc.sync.dma_start(out=mask_sb, in_=maskst[:, :, :])

            y1 = [acts.tile([128, R], F16, tag=f"y_{m}", name=f"y1_{m}")
                  for m in range(NK)]
            _attention(nc, pools, xq_sb, xt_sb, xq_sb, wunits, wv_view, 0,
                       bias_cols["bq1"], bias_cols["bk1"],
                       bvrows[:, 0:1024], bias_cols["bo1"], mask_sb, y1, ones128)
            enc_sb = [acts.tile([128, S], F16, tag=f"env_{k}", name=f"enc_{k}")
                      for k in range(NK)]
            for k in range(NK):
                nc.sync.dma_start(out=enc_sb[k], in_=enc[k * 128:(k + 1) * 128, :])
            o1 = [acts.tile([128, R], F16, tag=f"xq_{m}", name=f"o1_{m}")
                  for m in range(NK)]
            _ln(nc, pools, y1, bias_cols["g1"], bias_cols["be1"], o1, ones128)

            o2 = [pools["o2p"].tile([128, R], F16, tag=f"o2_{m}", name=f"o2_{m}")
                  for m in range(NK)]
            y2 = [acts.tile([128, R], F16, tag=f"y_{m}", name=f"y2_{m}")
                  for m in range(NK)]
            _attention(nc, pools, o1, enc_sb, o1, wunits, wv_view, 32,
                       bias_cols["bq2"], bias_cols["bk2"],
                       bvrows[:, 1024:2048], bias_cols["bo2"], None, y2, ones128)
            _ln(nc, pools, y2, bias_cols["g2"], bias_cols["be2"], o2, ones128)

        # ================= FFN scope =================
        with ExitStack() as fctx:
            hpool = fctx.enter_context(tc.tile_pool(name="hpool", bufs=1))
            facts = fctx.enter_context(tc.tile_pool(name="facts", bufs=1))
            o3p = fctx.enter_context(tc.tile_pool(name="o3pool", bufs=2))

            h = []
            for m in range(NM2):
                wt = pools["w"].tile([128, NK, 128], F16, tag="wcb",
                                     name=f"wff1_{m}")
                nc.sync.dma_start(
                    out=wt, in_=wunits[64 + m].rearrange("p (k c) -> p k c", c=128))
                psh = pools["psum"].tile([128, R], F32, tag="ps_gen", name="ps_h")
                for k in range(NK):
                    nc.tensor.matmul(psh, wt[:, k, :], o2[k],
                                     start=(k == 0), stop=(k == NK - 1))
                h_m = hpool.tile([128, R], F16, tag=f"h_{m}", name=f"h_{m}")
                nc.scalar.activation(h_m, psh, AF.Relu, bias=bff1c[:, m:m + 1])
                h.append(h_m)

            y3 = [facts.tile([128, R], F16, tag=f"y3_{m}", name=f"y3_{m}")
                  for m in range(NK)]
            for m in range(NK):
                psf = pools["psum"].tile([128, R], F32, tag="ps_gen", name="ps_f")
                for q in range(NM2 // NK):
                    wt = pools["w"].tile([128, NK, 128], F16, tag="wcb",
                                         name=f"wff2_{m}_{q}")
                    nc.sync.dma_start(
                        out=wt, in_=wunits[96 + m * 4 + q].rearrange(
                            "p (k c) -> p k c", c=128))
                    for k in range(NK):
                        nc.tensor.matmul(psf, wt[:, k, :], h[q * NK + k],
                                         start=(q == 0 and k == 0),
                                         stop=(q == NM2 // NK - 1 and k == NK - 1))
                nc.vector.scalar_tensor_tensor(y3[m], psf,
                                               bias_cols["b_ff2"][:, m:m + 1],
                                               o2[m], ALU.add, ALU.add)
            o3 = [o3p.tile([128, R], F32, tag="o3", name=f"o3_{m}")
                  for m in range(NK)]
            _ln(nc, pools, y3, bias_cols["g3"], bias_cols["be3"], o3, ones128)
            for m in range(NK):
                nc.sync.dma_start(out=out_t[m * 128:(m + 1) * 128, :], in_=o3[m])

    nc.compile()
    return nc


def _get_nc():
    if "nc" not in _NC_CACHE:
        _NC_CACHE["nc"] = build_nc()
    return _NC_CACHE["nc"]


def _tile4(w, c):
    """[D1, D2] -> [m, p, k, c] slabs: w[(k p), (m c)] -> [m][p][k][c]."""
    d1, d2 = w.shape
    return np.ascontiguousarray(
        w.reshape(d1 // 128, 128, d2 // c, c).transpose(2, 1, 0, 3))


def _pack_weights(inputs):
    """Concatenate all weights, pre-tiled into the exact slab layout the
    kernel DMAs: attn{1,2}: wv [4,128,8,256], wq/wk/wo [8,128,8,128];
    then w_ff1 [32,128,8,128]; then w_ff2 [8,4,128,8,128] ([m][q] major)."""
    parts = []
    for i in (1, 2):
        parts.append(_tile4(np.asarray(inputs[f"wv{i}"], np.float32), 256))
        parts.append(_tile4(np.asarray(inputs[f"wq{i}"], np.float32), 128))
        parts.append(_tile4(np.asarray(inputs[f"wk{i}"], np.float32), 128))
        parts.append(_tile4(np.asarray(inputs[f"wo{i}"], np.float32), 128))
    parts.append(_tile4(np.asarray(inputs["w_ff1"], np.float32), 128))
    wf2 = np.asarray(inputs["w_ff2"], np.float32).reshape(4, 8, 128, 8, 128)
    parts.append(np.ascontiguousarray(wf2.transpose(3, 0, 2, 1, 4)))
    return np.concatenate([p.reshape(-1) for p in parts]).astype(np.float16)


def _pack_biases(inputs):
    bias2d = np.zeros((128, BIAS_W), dtype=np.float32)
    for nm in _BCOLS:
        bias2d[:, _BOFF[nm]:_BOFF[nm] + 8] = np.asarray(
            inputs[nm], np.float32).reshape(8, 128).T
    bias2d[:, _BOFF["b_ff1"]:_BOFF["b_ff1"] + 32] = np.asarray(
        inputs["b_ff1"], np.float32).reshape(32, 128).T
    return np.ascontiguousarray(bias2d.reshape(-1))


def _pack_bv(inputs):
    return np.concatenate([np.asarray(inputs["bv1"], np.float32),
                           np.asarray(inputs["bv2"], np.float32)]).astype(
        np.float16).reshape(1, 2048)


def _make_in_maps(inputs):
    full_k = np.arange(S)
    wpack = _pack_weights(inputs)
    biaspack = _pack_biases(inputs)
    bvpack = _pack_bv(inputs)
    in_maps = []
    metas = []
    for c in range(8):
        b, half = c // 2, c % 2
        qidx = np.concatenate([np.arange(128) + 128 * blk for blk in BLOCKS[half]])
        xt_b = np.ascontiguousarray(
            np.asarray(inputs["inputs"][b], np.float32).T.astype(np.float16))
        enc_b = np.ascontiguousarray(
            np.asarray(inputs["enc_outputs"][b], np.float32).T.astype(np.float16))
        xq_b = np.ascontiguousarray(xt_b[:, qidx])
        # mask blocks: for s-tile s the kernel masks cols [c0, c0+128) with
        # mask[k in tile s, q = qidx[c0:c0+128]]; layout [k 128][s 8][q 128]
        mask = np.where(full_k[:, None] <= qidx[None, :], 0.0, NEG).astype(np.float16)
        mask_bl = np.zeros((128, NK, 128), dtype=np.float16)
        for s in range(NK):
            c0 = 128 * (s // 2)
            mask_bl[:, s, :] = mask[s * 128:(s + 1) * 128, c0:c0 + 128]
        m = {"xt": xt_b, "xq": xq_b, "enc": enc_b,
             "maskst": np.ascontiguousarray(mask_bl),
             "wpack": wpack, "biaspack": biaspack, "bvpack": bvpack}
        in_maps.append(m)
        metas.append((b, qidx))
    return in_maps, metas


def kernel(**inputs):
    nc = _get_nc()
    in_maps, metas = _make_in_maps(inputs)
    res = run_bass_kernel_spmd(nc, in_maps, core_ids=list(range(8)))
    out = np.zeros((B, S, D), dtype=np.float32)
    for c, (b, qidx) in enumerate(metas):
        out[b, qidx, :] = res.results[c]["out_t"].T
    return out


# revision 18
# speedup vs baseline: 1.2099x; 1.2099x over previous
"""Fused transformer decoder layer (self-attn + cross-attn + FFN, 3 LayerNorms)
for Trainium2, SPMD across 8 NeuronCores.

Sharding: 2 cores per batch element (B=4). Each core owns 512 query rows of
its batch element, picked as four 128-row blocks interleaved so the causal
self-attention work is balanced across the pair ({0,3,4,7} / {1,2,5,6}).
K/V projections are computed redundantly per core (no collectives needed).

On-device layout: activations are kept feature-major ("transposed", [D, rows])
so every linear layer uses the weight matrices exactly as stored:
    out^T [Dout, r] = matmul(lhsT=W[D, Dout]-tile, rhs=act^T[D, r]-tile).
Attention scores are computed transposed (scores^T[k, q] = K·Q^T per head);
softmax denominators are harvested by augmenting V with 64 all-ones columns,
which lands the per-query sums partition-replicated next to the attnV output.

v2: everything streams as bf16 (activations, weights, probs); PSUM/LN-stats/
softmax-normalization stay fp32. All weights are packed host-side into ONE
bf16 DRAM tensor laid out in the exact [slab, partition, free] tile order the
kernel consumes (contiguous DMA slabs), and all bias/gamma vectors into one
fp32 tensor: 6 kernel operands total, 2 orders of magnitude fewer startup
DMA issues than v1.
"""

import numpy as np

import concourse.bacc as bacc
import concourse.bass as bass
import concourse.mybir as mybir
import concourse.tile as tile
from concourse.bass_utils import run_bass_kernel_spmd

F32 = mybir.dt.float32
F16 = mybir.dt.float16
AF = mybir.ActivationFunctionType
ALU = mybir.AluOpType

B, S, D, DFF, H = 4, 1024, 1024, 4096, 16
R = 512                   # query rows per core
NK = D // 128             # 8 k-tiles over D
NP = H // 2               # 8 head pairs
NM2 = DFF // 128          # 32 m-tiles over DFF
EPS = 1e-3
NEG = -30000.0
BLOCKS = [[0, 3, 4, 7], [1, 2, 5, 6]]   # 128-row q-blocks per half-core

# wpack unit layout: 128 units of [128, 1024] bf16 (see _pack_weights)
#   units  0..31 : attn1  (wv 8u, wq 8u, wk 8u, wo 8u)
#   units 32..63 : attn2  (same)
#   units 64..95 : w_ff1  (32 slabs)
#   units 96..127: w_ff2  (32 slabs, [m][q] order)
NU = 128

# biaspack columns (fp32 [128, 136]):
_BCOLS = ("bq1", "bk1", "bo1", "bq2", "bk2", "bo2", "b_ff2",
          "g1", "be1", "g2", "be2", "g3", "be3")
_BOFF = {nm: 8 * i for i, nm in enumerate(_BCOLS)}
_BOFF["b_ff1"] = 8 * len(_BCOLS)          # 32 cols
BIAS_W = 136  # 13*8 + 32

_NC_CACHE = {}


def _ln(nc, pools, y, gcol, bcol, out_tiles, ones128):
    """LayerNorm over the partition (feature) axis of 8 [128, R] tiles."""
    ps, tmp = pools["psum"], pools["lntmp"]
    pssum = ps.tile([128, R], F32, tag="ps_gen", name="ln_ps_sum")
    for m in range(NK):
        nc.tensor.matmul(pssum, ones128, y[m], start=(m == 0), stop=(m == NK - 1))
    pssq = ps.tile([128, R], F32, tag="ps_gen", name="ln_ps_sq")
    for m in range(NK):
        sq = pools["sq"].tile([128, R], F16, tag="ln_sqt", name="ln_sqt")
        nc.scalar.activation(sq, y[m], AF.Square)
        nc.tensor.matmul(pssq, ones128, sq, start=(m == 0), stop=(m == NK - 1))
    mean = tmp.tile([128, R], F32, tag="ln_mean", name="ln_mean")
    nc.vector.tensor_scalar_mul(mean, pssum, 1.0 / D)
    rv = tmp.tile([128, R], F32, tag="ln_sc", name="ln_rv")
    nc.vector.tensor_scalar_mul(rv, pssq, 1.0 / D)      # E[x^2]
    msq = tmp.tile([128, R], F32, tag="ln_t", name="ln_msq")
    nc.vector.tensor_mul(msq, mean, mean)
    nc.vector.tensor_sub(rv, rv, msq)                   # var
    nc.scalar.activation(rv, rv, AF.Sqrt, bias=pools["epsc"][:, 0:1])
    nc.vector.reciprocal(rv, rv)                        # rstd (broadcast)
    nc.vector.tensor_mul(mean, mean, rv)                # mean*rstd (broadcast)
    for m in range(NK):
        t = tmp.tile([128, R], F32, tag="ln_t", name="ln_t")
        nc.vector.tensor_mul(t, y[m], rv)
        nc.vector.tensor_sub(t, t, mean)
        nc.vector.tensor_scalar(out_tiles[m], t, gcol[:, m:m + 1], bcol[:, m:m + 1],
                                ALU.mult, ALU.add)


def _attention(nc, pools, q_src, kv_src, resid, wunits, wv_view, u0,
               bqc, bkc, bvrow, boc, mask_sb, y_out, ones_row):
    """One multi-head attention block + residual; writes pre-LN y_out tiles.

    wunits: [NU, 128, 1024] bf16 view of wpack; wv_view: [NU//2, 128, 2048]
    view of the same; u0: first unit of this attention's 32-unit region
    (wv 8u, wq 8u, wk 8u, wo 8u)."""
    wpool, ps, tmp = pools["w"], pools["psum"], pools["atmp"]
    causal = mask_sb is not None
    uq, uk, uo = u0 + 8, u0 + 16, u0 + 24

    # persistent V_aug pair; ones columns written once per attention
    vaugs = [pools["vaug"].tile([128, NK, 256], F16, tag="vaug",
                                name=f"vaug_{i}") for i in range(2)]
    for i in range(2):
        nc.vector.memset(vaugs[i][:, :, 64:192], 1.0)

    nh = []
    for p in range(NP):
        # ---- V for two pairs produced together ----
        # V_aug[:, s, :]: cols [v_h0(64) | ones(128) | v_h1(64)]
        if p % 2 == 0:
            j = p // 2
            wvb = wpool.tile([128, NK, 256], F16, tag="wvb", name=f"wvb_{j}")
            nc.sync.dma_start(
                out=wvb,
                in_=wv_view[u0 // 2 + j].rearrange("p (k c) -> p k c", c=256))
            for s in range(NK):
                psv = ps.tile([128, 256], F32, tag="ps_gen", name="ps_v")
                for k in range(NK):
                    nc.tensor.matmul(psv, kv_src[k][:, s * 128:(s + 1) * 128],
                                     wvb[:, k, :],
                                     start=(k == 0), stop=False)
                # bias row via K=1 matmul: psv[m, c] += 1 * bv[c]
                nc.tensor.matmul(psv, ones_row[0:1, :],
                                 bvrow[0:1, j * 256:(j + 1) * 256],
                                 start=False, stop=True)
                for i in range(2):
                    dst = vaugs[i][:, s, :].rearrange(
                        "p (a c) -> p a c", c=64)[:, 0:4:3, :]
                    srcp = psv[:, i * 128:(i + 1) * 128].rearrange(
                        "p (a c) -> p a c", c=64)
                    nc.vector.tensor_copy(dst, srcp)
        vaug = vaugs[p % 2]

        # ---- K^T and Q^T for this pair only (pipelines across pairs) ----
        wcb = wpool.tile([128, NK, 128], F16, tag="wcb", name=f"wkc_{p}")
        nc.sync.dma_start(out=wcb,
                          in_=wunits[uk + p].rearrange("p (k c) -> p k c", c=128))
        kt_p = pools["kt"].tile([128, S], F16, tag="kt", name=f"kt_{p}")
        for half in range(2):
            psk = ps.tile([128, R], F32, tag="ps_gen", name="ps_k")
            for k in range(NK):
                nc.tensor.matmul(psk, wcb[:, k, :],
                                 kv_src[k][:, half * 512:(half + 1) * 512],
                                 start=(k == 0), stop=(k == NK - 1))
            nc.scalar.activation(kt_p[:, half * 512:(half + 1) * 512], psk,
                                 AF.Identity, bias=bkc[:, p:p + 1])
        wcb = wpool.tile([128, NK, 128], F16, tag="wcb", name=f"wqc_{p}")
        nc.sync.dma_start(out=wcb,
                          in_=wunits[uq + p].rearrange("p (k c) -> p k c", c=128))
        psq = ps.tile([128, R], F32, tag="ps_gen", name="ps_q")
        for k in range(NK):
            nc.tensor.matmul(psq, wcb[:, k, :], q_src[k],
                             start=(k == 0), stop=(k == NK - 1))
        qt_p = pools["qt"].tile([128, R], F16, tag="qt", name=f"qt_{p}")
        nc.scalar.activation(qt_p, psq, AF.Identity, bias=bqc[:, p:p + 1])

        # ---- scores^T, exp, attnV (softmax sums ride along in V_aug ones) ----
        psa0 = pools["psatt"].tile([128, R], F32, tag="ps_att0", name="ps_att0")
        psa1 = pools["psatt"].tile([128, R], F32, tag="ps_att1", name="ps_att1")
        for s in range(NK):
            c0 = 128 * (s // 2) if causal else 0
            n = R - c0
            ssc0 = pools["pssc"].tile([128, R], F32, tag="ps_sc0", name="ps_sc0")
            ssc1 = pools["pssc"].tile([128, R], F32, tag="ps_sc1", name="ps_sc1")
            nc.tensor.matmul(ssc0[:, 0:n], kt_p[0:64, s * 128:(s + 1) * 128],
                             qt_p[0:64, c0:R], start=True, stop=True)
            nc.tensor.matmul(ssc1[:, 0:n], kt_p[64:128, s * 128:(s + 1) * 128],
                             qt_p[64:128, c0:R], start=True, stop=True)
            if causal:
                nc.vector.tensor_add(ssc0[:, 0:128], ssc0[:, 0:128], mask_sb[:, s, :])
                nc.vector.tensor_add(ssc1[:, 0:128], ssc1[:, 0:128], mask_sb[:, s, :])
            e0 = pools["exp"].tile([128, R], F16, tag="e0", name="e0")
            e1 = pools["exp"].tile([128, R], F16, tag="e1", name="e1")
            nc.scalar.activation(e0[:, 0:n], ssc0[:, 0:n], AF.Exp, scale=0.125)
            nc.scalar.activation(e1[:, 0:n], ssc1[:, 0:n], AF.Exp, scale=0.125)
            nc.tensor.matmul(psa0[:, c0:R], vaug[:, s, 0:128], e0[:, 0:n],
                             start=(s == 0), stop=(s == NK - 1), skip_group_check=True)
            nc.tensor.matmul(psa1[:, c0:R], vaug[:, s, 128:256], e1[:, 0:n],
                             start=(s == 0), stop=(s == NK - 1), skip_group_check=True)

        # normalize: head0 out rows 0:64 / sums 64:128; head1 sums 0:64 / out 64:128
        nh_p = pools["nh"].tile([128, R], F16, tag=f"nh_{p}", name=f"nh_{p}")
        inv = tmp.tile([128, R], F32, tag="inv", name="inv")
        invs = tmp.tile([128, R], F32, tag="invs", name="invs")
        nc.vector.reciprocal(inv[64:128, :], psa0[64:128, :])
        nc.sync.dma_start(out=invs[0:64, :], in_=inv[64:128, :])
        nc.vector.tensor_mul(nh_p[0:64, :], psa0[0:64, :], invs[0:64, :])
        nc.vector.reciprocal(inv[0:64, :], psa1[0:64, :])
        nc.sync.dma_start(out=invs[64:128, :], in_=inv[0:64, :])
        nc.vector.tensor_mul(nh_p[64:128, :], psa1[64:128, :], invs[64:128, :])
        nh.append(nh_p)

    # ---- output projection + bias + residual ----
    for m in range(NK):
        wcb = wpool.tile([128, NK, 128], F16, tag="wcb", name=f"woc_{m}")
        nc.sync.dma_start(out=wcb,
                          in_=wunits[uo + m].rearrange("p (k c) -> p k c", c=128))
        pso = ps.tile([128, R], F32, tag="ps_gen", name="ps_o")
        for p in range(NP):
            nc.tensor.matmul(pso, wcb[:, p, :], nh[p],
                             start=(p == 0), stop=(p == NP - 1))
        nc.vector.scalar_tensor_tensor(y_out[m], pso, boc[:, m:m + 1],
                                       resid[m], ALU.add, ALU.add)


def build_nc(repeat=1):
    nc = bacc.Bacc("TRN2", target_bir_lowering=False, debug=False)

    xt = nc.dram_tensor("xt", [D, S], F16, kind="ExternalInput")
    xq = nc.dram_tensor("xq", [D, R], F16, kind="ExternalInput")
    enc = nc.dram_tensor("enc", [D, S], F16, kind="ExternalInput")
    maskst = nc.dram_tensor("maskst", [128, NK, 128], F16, kind="ExternalInput")
    wpack = nc.dram_tensor("wpack", [NU * 128 * 1024], F16, kind="ExternalInput")
    biaspack = nc.dram_tensor("biaspack", [128 * BIAS_W], F32,
                              kind="ExternalInput")
    bvpack = nc.dram_tensor("bvpack", [1, 2048], F16, kind="ExternalInput")
    out_t = nc.dram_tensor("out_t", [D, R], F32, kind="ExternalOutput")

    wunits = wpack.rearrange("(u p x) -> u p x", p=128, x=1024)
    wv_view = wpack.rearrange("(v p y) -> v p y", p=128, y=2048)
    bias2d = biaspack.rearrange("(p w) -> p w", w=BIAS_W)

    from contextlib import ExitStack, nullcontext
    with tile.TileContext(nc) as tc, ExitStack() as ctx:
        pools = {
            "const": ctx.enter_context(tc.tile_pool(name="const", bufs=1)),
            "w": ctx.enter_context(tc.tile_pool(name="wpool", bufs=3)),
            "psum": ctx.enter_context(tc.tile_pool(name="pspool", bufs=2, space="PSUM")),
            "lntmp": ctx.enter_context(tc.tile_pool(name="lntmp", bufs=1)),
            "sq": ctx.enter_context(tc.tile_pool(name="sqpool", bufs=2)),
            "o2p": ctx.enter_context(tc.tile_pool(name="o2pool", bufs=1)),
        }
        for nm, bufs, space in (("acts", 1, "SBUF"), ("qt", 2, "SBUF"),
                                ("kt", 2, "SBUF"), ("vaug", 2, "SBUF"),
                                ("nh", 1, "SBUF"), ("exp", 2, "SBUF"),
                                ("atmp", 1, "SBUF"), ("amask", 1, "SBUF"),
                                ("pssc", 2, "PSUM"), ("psatt", 1, "PSUM"),
                                ("hpool", 1, "SBUF"), ("facts", 1, "SBUF"),
                                ("o3p", 2, "SBUF")):
            pools[nm] = ctx.enter_context(
                tc.tile_pool(name=nm, bufs=bufs, space=space))
        const = pools["const"]
        acts = pools["acts"]
        hpool, facts, o3p = pools["hpool"], pools["facts"], pools["o3p"]

        with (tc.For_i(0, repeat, 1) if repeat > 1 else nullcontext()):
            # activations first: these gate the first matmuls
            xt_sb = [acts.tile([128, S], F16, tag=f"kv_{k}", name=f"xt_{k}")
                     for k in range(NK)]
            xq_sb = [acts.tile([128, R], F16, tag=f"xq_{k}", name=f"xq_{k}")
                     for k in range(NK)]
            for k in range(NK):
                nc.sync.dma_start(out=xt_sb[k], in_=xt[k * 128:(k + 1) * 128, :])
            for k in range(NK):
                nc.sync.dma_start(out=xq_sb[k], in_=xq[k * 128:(k + 1) * 128, :])

            # small constants after the big activation loads
            ones128 = const.tile([128, 128], F16, tag="ones128", name="ones128")
            nc.vector.memset(ones128, 1.0)
            epsc = const.tile([128, 1], F32, tag="epsc", name="epsc")
            nc.vector.memset(epsc, EPS)
            pools["epsc"] = epsc
            biastile = const.tile([128, BIAS_W], F32, tag="biast", name="biast")
            nc.sync.dma_start(out=biastile, in_=bias2d)
            bvrows = const.tile([1, 2048], F16, tag="bvrows", name="bvrows")
            nc.sync.dma_start(out=bvrows, in_=bvpack[:, :])
            bias_cols = {nm: biastile[:, _BOFF[nm]:_BOFF[nm] + 8] for nm in _BCOLS}
            bff1c = biastile[:, _BOFF["b_ff1"]:_BOFF["b_ff1"] + 32]

            mask_sb = pools["amask"].tile([128, NK, 128], F16, tag="mask", name="mask")
            nc.sync.dma_start(out=mask_sb, in_=maskst[:, :, :])

            y1 = [acts.tile([128, R], F16, tag=f"y_{m}", name=f"y1_{m}")
                  for m in range(NK)]
            _attention(nc, pools, xq_sb, xt_sb, xq_sb, wunits, wv_view, 0,
                       bias_cols["bq1"], bias_cols["bk1"],
                       bvrows[:, 0:1024], bias_cols["bo1"], mask_sb, y1, ones128)
            enc_sb = [acts.tile([128, S], F16, tag=f"env_{k}", name=f"enc_{k}")
                      for k in range(NK)]
            for k in range(NK):
                nc.sync.dma_start(out=enc_sb[k], in_=enc[k * 128:(k + 1) * 128, :])
            o1 = [acts.tile([128, R], F16, tag=f"xq_{m}", name=f"o1_{m}")
                  for m in range(NK)]
            _ln(nc, pools, y1, bias_cols["g1"], bias_cols["be1"], o1, ones128)

            o2 = [pools["o2p"].tile([128, R], F16, tag=f"o2_{m}", name=f"o2_{m}")
                  for m in range(NK)]
            y2 = [acts.tile([128, R], F16, tag=f"y_{m}", name=f"y2_{m}")
                  for m in range(NK)]
            _attention(nc, pools, o1, enc_sb, o1, wunits, wv_view, 32,
                       bias_cols["bq2"], bias_cols["bk2"],
                       bvrows[:, 1024:2048], bias_cols["bo2"], None, y2, ones128)
            _ln(nc, pools, y2, bias_cols["g2"], bias_cols["be2"], o2, ones128)

            # ================= FFN =================
            h = []
            for m in range(NM2):
                wt = pools["w"].tile([128, NK, 128], F16, tag="wcb",
                                     name=f"wff1_{m}")
                nc.sync.dma_start(
                    out=wt, in_=wunits[64 + m].rearrange("p (k c) -> p k c", c=128))
                psh = pools["psum"].tile([128, R], F32, tag="ps_gen", name="ps_h")
                for k in range(NK):
                    nc.tensor.matmul(psh, wt[:, k, :], o2[k],
                                     start=(k == 0), stop=(k == NK - 1))
                h_m = hpool.tile([128, R], F16, tag=f"h_{m}", name=f"h_{m}")
                nc.scalar.activation(h_m, psh, AF.Relu, bias=bff1c[:, m:m + 1])
                h.append(h_m)

            y3 = [facts.tile([128, R], F16, tag=f"y3_{m}", name=f"y3_{m}")
                  for m in range(NK)]
            for m in range(NK):
                psf = pools["psum"].tile([128, R], F32, tag="ps_gen", name="ps_f")
                for q in range(NM2 // NK):
                    wt = pools["w"].tile([128, NK, 128], F16, tag="wcb",
                                         name=f"wff2_{m}_{q}")
                    nc.sync.dma_start(
                        out=wt, in_=wunits[96 + m * 4 + q].rearrange(
                            "p (k c) -> p k c", c=128))
                    for k in range(NK):
                        nc.tensor.matmul(psf, wt[:, k, :], h[q * NK + k],
                                         start=(q == 0 and k == 0),
                                         stop=(q == NM2 // NK - 1 and k == NK - 1))
                nc.vector.scalar_tensor_tensor(y3[m], psf,
                                               bias_cols["b_ff2"][:, m:m + 1],
                                               o2[m], ALU.add, ALU.add)
            o3 = [o3p.tile([128, R], F32, tag="o3", name=f"o3_{m}")
                  for m in range(NK)]
            _ln(nc, pools, y3, bias_cols["g3"], bias_cols["be3"], o3, ones128)
            for m in range(NK):
                nc.sync.dma_start(out=out_t[m * 128:(m + 1) * 128, :], in_=o3[m])

    nc.compile()
    return nc


def _get_nc(repeat=1):
    if repeat not in _NC_CACHE:
        _NC_CACHE[repeat] = build_nc(repeat)
    return _NC_CACHE[repeat]


def _tile4(w, c):
    """[D1, D2] -> [m, p, k, c] slabs: w[(k p), (m c)] -> [m][p][k][c]."""
    d1, d2 = w.shape
    return np.ascontiguousarray(
        w.reshape(d1 // 128, 128, d2 // c, c).transpose(2, 1, 0, 3))


def _pack_weights(inputs):
    """Concatenate all weights, pre-tiled into the exact slab layout the
    kernel DMAs: attn{1,2}: wv [4,128,8,256], wq/wk/wo [8,128,8,128];
    then w_ff1 [32,128,8,128]; then w_ff2 [8,4,128,8,128] ([m][q] major)."""
    parts = []
    for i in (1, 2):
        parts.append(_tile4(np.asarray(inputs[f"wv{i}"], np.float32), 256))
        parts.append(_tile4(np.asarray(inputs[f"wq{i}"], np.float32), 128))
        parts.append(_tile4(np.asarray(inputs[f"wk{i}"], np.float32), 128))
        parts.append(_tile4(np.asarray(inputs[f"wo{i}"], np.float32), 128))
    parts.append(_tile4(np.asarray(inputs["w_ff1"], np.float32), 128))
    wf2 = np.asarray(inputs["w_ff2"], np.float32).reshape(4, 8, 128, 8, 128)
    parts.append(np.ascontiguousarray(wf2.transpose(3, 0, 2, 1, 4)))
    return np.concatenate([p.reshape(-1) for p in parts]).astype(np.float16)


def _pack_biases(inputs):
    bias2d = np.zeros((128, BIAS_W), dtype=np.float32)
    for nm in _BCOLS:
        bias2d[:, _BOFF[nm]:_BOFF[nm] + 8] = np.asarray(
            inputs[nm], np.float32).reshape(8, 128).T
    bias2d[:, _BOFF["b_ff1"]:_BOFF["b_ff1"] + 32] = np.asarray(
        inputs["b_ff1"], np.float32).reshape(32, 128).T
    return np.ascontiguousarray(bias2d.reshape(-1))


def _pack_bv(inputs):
    return np.concatenate([np.asarray(inputs["bv1"], np.float32),
                           np.asarray(inputs["bv2"], np.float32)]).astype(
        np.float16).reshape(1, 2048)


def _make_in_maps(inputs):
    full_k = np.arange(S)
    wpack = _pack_weights(inputs)
    biaspack = _pack_biases(inputs)
    bvpack = _pack_bv(inputs)
    in_maps = []
    metas = []
    for c in range(8):
        b, half = c // 2, c % 2
        qidx = np.concatenate([np.arange(128) + 128 * blk for blk in BLOCKS[half]])
        xt_b = np.ascontiguousarray(
            np.asarray(inputs["inputs"][b], np.float32).T.astype(np.float16))
        enc_b = np.ascontiguousarray(
            np.asarray(inputs["enc_outputs"][b], np.float32).T.astype(np.float16))
        xq_b = np.ascontiguousarray(xt_b[:, qidx])
        # mask blocks: for s-tile s the kernel masks cols [c0, c0+128) with
        # mask[k in tile s, q = qidx[c0:c0+128]]; layout [k 128][s 8][q 128]
        mask = np.where(full_k[:, None] <= qidx[None, :], 0.0, NEG).astype(np.float16)
        mask_bl = np.zeros((128, NK, 128), dtype=np.float16)
        for s in range(NK):
            c0 = 128 * (s // 2)
            mask_bl[:, s, :] = mask[s * 128:(s + 1) * 128, c0:c0 + 128]
        m = {"xt": xt_b, "xq": xq_b, "enc": enc_b,
             "maskst": np.ascontiguousarray(mask_bl),
             "wpack": wpack, "biaspack": biaspack, "bvpack": bvpack}
        in_maps.append(m)
        metas.append((b, qidx))
    return in_maps, metas


def kernel(**inputs):
    nc = _get_nc()
    in_maps, metas = _make_in_maps(inputs)
    res = run_bass_kernel_spmd(nc, in_maps, core_ids=list(range(8)))
    out = np.zeros((B, S, D), dtype=np.float32)
    for c, (b, qidx) in enumerate(metas):
        out[b, qidx, :] = res.results[c]["out_t"].T
    return out
